# revision 2
# baseline (speedup 1.0000x reference)
"""DeepSeek-MoE (64 experts, top-6 grouped routing) on 8 TRN2 NeuronCores.

Expert-parallel, v2: all dispatch/combine is done with one-hot MATMULS on
the PE — no indirect DMA at all (v1's dma_gather / dma_scatter_add /
per-column indirect scatters generated ~40K software-DGE descriptors which
dominated hardware time at ~µs each).

  - Every core receives the full hidden_states (fp32 rearranged for the
    router, fp16 row-major for the expert GEMMs) plus an 8-expert shard of
    w_gate/w_up/w_down (fp16) and a group-rotated gate matrix so that its
    local experts always occupy routing columns 0..7.
  - Router (fp32): logits -> softmax -> grouped top-6 -> renormalized
    combine weights CL, mask M, and per-(token,expert) slot index pos
    (prefix-sum matmuls).
  - Gather: for each local expert e, one-hot O_tt[t, s] = (pos[t]==s) is
    built on the DVE and XgT[h, s] = sum_t x[t, h] * O[t, s] accumulates on
    the PE straight into the [H-part, slot] layout the MLP wants.
  - MLP: fp16 matmuls with fp32 PSUM accumulation (gate/up -> silu*up ->
    down), producing D_e[slot, h] fp16. Combine weights are NOT folded here.
  - Combine: OT_w[s, t] = (pos[t]==s) * CL[t] is built per token tile from
    broadcast rows, and Y[t, h] = sum_s OT_w[s, t] * D[s, h] accumulates on
    the PE over all 8 experts; y is written densely (no scatter).
  - Host sums the 8 partial outputs.

Empty slots produce exact zeros end-to-end (O column zero -> Xg zero ->
silu(0)*0 = 0 -> D zero), and unrouted tokens carry pos=100000 which never
matches a slot, so no counts/registers/bounds checks are needed.
"""

import os

import numpy as np

import concourse.bacc as bacc
import concourse.bass as bass
import concourse.mybir as mybir
import concourse.tile as tile
from concourse.bass_utils import run_bass_kernel_spmd
from concourse.masks import make_identity, make_upper_triangular

P = 128
T = 4096          # tokens
H = 2048          # hidden
ID = 1408         # intermediate
E = 64            # experts
EL = 8            # local experts per core
NCORES = 8
CAP = 512         # per-expert token capacity (actual max count is ~454)
TT = T // P       # 32 token tiles
HC = H // P       # 16 hidden chunks
IC = ID // P      # 11 intermediate chunks
HB = H // 512     # 4 hidden blocks (down-proj rhs width 512)
SB = CAP // P     # 4 slot blocks per expert
HG = 8            # gather h-groups (2 psum banks each)
HH = HC // 2      # half of the hidden chunks (gate/up weight half-tiles)
BIGF = 30000.0    # unrouted-token pos marker: never matches a slot
                  # (finite in fp16: 0*inf=NaN would poison the
                  #  sel8 broadcast matmul over pos_rows)

f32 = mybir.dt.float32
f16 = mybir.dt.float16
i32 = mybir.dt.int32
u8 = mybir.dt.uint8
AF = mybir.ActivationFunctionType
OP = mybir.AluOpType
AX = mybir.AxisListType


def build_nc(debug=False, sim_safe=False, debug_out=False):
    nc = bacc.Bacc("TRN2", target_bir_lowering=False, debug=debug)

    dbg = {}
    if debug_out:
        dbg["pos"] = nc.dram_tensor("dbg_pos", [P, TT, EL], f32,
                                    kind="ExternalOutput")
        dbg["cl"] = nc.dram_tensor("dbg_cl", [P, TT, EL], f32,
                                   kind="ExternalOutput")
        dbg["posrows"] = nc.dram_tensor("dbg_posrows", [EL, T], f16,
                                        kind="ExternalOutput")
        dbg["clrows"] = nc.dram_tensor("dbg_clrows", [EL, T], f16,
                                       kind="ExternalOutput")
        dbg["iotas"] = nc.dram_tensor("dbg_iotas", [P, CAP], f16,
                                      kind="ExternalOutput")
        dbg["sel8"] = nc.dram_tensor("dbg_sel8", [EL, EL, P], f16,
                                     kind="ExternalOutput")
        dbg["o0"] = nc.dram_tensor("dbg_o0", [P, CAP], f16,
                                   kind="ExternalOutput")
        dbg["xgt0"] = nc.dram_tensor("dbg_xgt0", [P, HC, CAP], f16,
                                     kind="ExternalOutput")
        dbg["d0"] = nc.dram_tensor("dbg_d0", [P, SB, H], f16,
                                   kind="ExternalOutput")
        dbg["ot0"] = nc.dram_tensor("dbg_ot0", [P, SB, P], f16,
                                    kind="ExternalOutput")

    x16 = nc.dram_tensor("x16", [T, H], f16, kind="ExternalInput")
    xr = nc.dram_tensor("xr", [TT, P, HC, P], f32, kind="ExternalInput")
    gwt = nc.dram_tensor("gwt", [P, HC, E], f32, kind="ExternalInput")
    wg = nc.dram_tensor("wg", [EL, IC, P, HC, P], f16, kind="ExternalInput")
    wu = nc.dram_tensor("wu", [EL, IC, P, HC, P], f16, kind="ExternalInput")
    wd = nc.dram_tensor("wd", [EL, HB, P, IC, 512], f16, kind="ExternalInput")
    y = nc.dram_tensor("y", [T, H], f32, kind="ExternalOutput")

    with tile.TileContext(nc) as tc:
        with tc.tile_pool(name="const", bufs=1) as cp:
            ident = cp.tile([P, P], f32)
            make_identity(nc, ident[:])
            ut = cp.tile([P, P], f32)
            make_upper_triangular(nc, ut[:], val=1.0, diag=True)
            sut = cp.tile([TT, TT], f32)
            make_upper_triangular(nc, sut[:], val=1.0, diag=False)
            onesk = cp.tile([P, 1], f32)
            nc.vector.memset(onesk[:], 1.0)
            ones1 = cp.tile([1, P], f32)
            nc.vector.memset(ones1[:], 1.0)
            M_all = cp.tile([P, TT, EL], f32)
            CL_all = cp.tile([P, TT, EL], f32)     # combine weights
            offs_flat = cp.tile([1, TT * EL], f32)
            tot32 = cp.tile([TT, EL], f32)
            # iota row: [p, s] = s, fp16
            iota_s_i = cp.tile([P, CAP], i32)
            nc.gpsimd.iota(iota_s_i[:], pattern=[[1, CAP]], base=0,
                           channel_multiplier=0)
            iota_s = cp.tile([P, CAP], f16)
            nc.vector.tensor_copy(iota_s[:], iota_s_i[:])
            # iota cols: [p, sc] = p + 128*sc, fp16
            iota4_i = cp.tile([P, SB], i32)
            nc.gpsimd.iota(iota4_i[:], pattern=[[P, SB]], base=0,
                           channel_multiplier=1)
            iota4 = cp.tile([P, SB], f32)
            nc.vector.tensor_copy(iota4[:], iota4_i[:])
            # sel8[p, e, i] = 1 iff p == e (expert-row selector for
            # broadcast matmuls from the packed row tiles)
            iota_e_i = cp.tile([EL, EL], i32)
            nc.gpsimd.iota(iota_e_i[:], pattern=[[1, EL]], base=0,
                           channel_multiplier=0)
            iota_e = cp.tile([EL, EL], f16)
            nc.vector.tensor_copy(iota_e[:], iota_e_i[:])
            iota_p_i = cp.tile([EL, 1], i32)
            nc.gpsimd.iota(iota_p_i[:], pattern=[[0, 1]], base=0,
                           channel_multiplier=1)
            iota_p = cp.tile([EL, 1], f32)
            nc.vector.tensor_copy(iota_p[:], iota_p_i[:])
            sel8 = cp.tile([EL, EL, P], f16)
            nc.vector.tensor_copy(
                sel8[:], iota_e[:, :, None].to_broadcast([EL, EL, P]))
            nc.vector.tensor_scalar(sel8[:], sel8[:], iota_p[:, 0:1],
                                    scalar2=None, op0=OP.is_equal)

            # ---------------- Phase A: router over all 32 token tiles
            with tc.tile_pool(name="ra", bufs=3) as ra, \
                 tc.tile_pool(name="rg", bufs=1) as rg, \
                 tc.tile_pool(name="rp", bufs=2, space="PSUM") as rp:
                gwt_sb = rg.tile([P, HC, E], f32)
                nc.sync.dma_start(gwt_sb[:], gwt[:])
                for tt in range(TT):
                    xrt = ra.tile([P, HC, P], f32, tag="xrt")
                    nc.sync.dma_start(xrt[:], xr[tt])
                    psl = rp.tile([P, E], f32, tag="psl")
                    for h in range(HC):
                        nc.tensor.matmul(psl[:], lhsT=xrt[:, h, :],
                                         rhs=gwt_sb[:, h, :],
                                         start=(h == 0), stop=(h == HC - 1))
                    nrm = ra.tile([P, 1], f32, tag="nrm")
                    nc.vector.tensor_reduce(out=nrm[:], in_=psl[:], axis=AX.X,
                                            op=OP.max, negate=True)
                    expt = ra.tile([P, E], f32, tag="expt")
                    nc.scalar.activation(expt[:], psl[:], AF.Exp, bias=nrm[:])
                    gs = ra.tile([P, 8], f32, tag="gs")
                    nc.vector.tensor_reduce(
                        out=gs[:], in_=expt[:].rearrange("p (g k) -> p g k", g=8),
                        axis=AX.X, op=OP.max)
                    g8 = ra.tile([P, 8], f32, tag="g8")
                    nc.vector.max(out=g8[:], in_=gs[:])
                    g3 = ra.tile([P, 8], f32, tag="g3")
                    nc.vector.tensor_copy(g3[:], g8[:])
                    nc.vector.memset(g3[:, 3:8], 0.0)
                    gsr = ra.tile([P, 8], f32, tag="gsr")
                    nc.vector.match_replace(out=gsr[:], in_to_replace=g3[:],
                                            in_values=gs[:], imm_value=0.0)
                    gm = ra.tile([P, 8], f32, tag="gm")
                    nc.vector.tensor_sub(gm[:], gs[:], gsr[:])
                    nc.vector.tensor_scalar(gm[:], gm[:], 0.0, scalar2=None,
                                            op0=OP.is_gt)
                    msk = ra.tile([P, E], f32, tag="msk")
                    nc.vector.tensor_tensor(
                        out=msk[:].rearrange("p (g k) -> p g k", g=8),
                        in0=expt[:].rearrange("p (g k) -> p g k", g=8),
                        in1=gm[:, :, None].to_broadcast([P, 8, 8]),
                        op=OP.mult)
                    m8 = ra.tile([P, 8], f32, tag="m8")
                    nc.vector.max(out=m8[:], in_=msk[:])
                    m6 = ra.tile([P, 8], f32, tag="m6")
                    nc.vector.tensor_copy(m6[:], m8[:])
                    nc.vector.memset(m6[:, 6:8], -1.0)
                    rem = ra.tile([P, E], f32, tag="rem")
                    nc.vector.match_replace(out=rem[:], in_to_replace=m6[:],
                                            in_values=msk[:], imm_value=0.0)
                    sel = ra.tile([P, E], f32, tag="sel")
                    nc.vector.tensor_sub(sel[:], msk[:], rem[:])
                    rs = ra.tile([P, 1], f32, tag="rs")
                    nc.vector.tensor_reduce(out=rs[:], in_=sel[:], axis=AX.X,
                                            op=OP.add)
                    nc.vector.tensor_scalar(rs[:], rs[:], 1e-20, scalar2=None,
                                            op0=OP.add)
                    rinv = ra.tile([P, 1], f32, tag="rinv")
                    nc.vector.reciprocal(rinv[:], rs[:])
                    cl = ra.tile([P, EL], f32, tag="cl")
                    nc.vector.tensor_scalar(cl[:], sel[:, 0:EL], rinv[:],
                                            scalar2=None, op0=OP.mult)
                    nc.vector.tensor_copy(CL_all[:, tt, :], cl[:])
                    nc.vector.tensor_scalar(M_all[:, tt, :], cl[:], 0.0,
                                            scalar2=None, op0=OP.is_gt)

            # ---------------- Phase B: totals -> per-tile offsets
            with tc.tile_pool(name="pb", bufs=1) as pb, \
                 tc.tile_pool(name="pbp", bufs=1, space="PSUM") as pbp:
                totp = pbp.tile([1, TT * EL], f32)
                nc.tensor.matmul(totp[:], lhsT=onesk[:],
                                 rhs=M_all[:].rearrange("p t e -> p (t e)"),
                                 start=True, stop=True)
                tots = pb.tile([1, TT * EL], f32)
                nc.vector.tensor_copy(tots[:], totp[:])
                nc.sync.dma_start(tot32[:], tots[:])
                offp = pbp.tile([TT, EL], f32)
                nc.tensor.matmul(offp[:], lhsT=sut[:], rhs=tot32[:],
                                 start=True, stop=True)
                offs32 = pb.tile([TT, EL], f32)
                nc.vector.tensor_copy(offs32[:], offp[:])
                nc.sync.dma_start(offs_flat[:], offs32[:])

            # ---------------- Phase C: slot index pos per (token, expert)
            pos_all = cp.tile([P, TT, EL], f32)
            with tc.tile_pool(name="pc", bufs=3) as pcp, \
                 tc.tile_pool(name="pcs", bufs=2, space="PSUM") as pcs:
                for tt in range(TT):
                    sp = pcs.tile([P, EL], f32, tag="sp")
                    nc.tensor.matmul(sp[:], lhsT=ut[:], rhs=M_all[:, tt, :],
                                     start=True, stop=False)
                    nc.tensor.matmul(sp[:], lhsT=ones1[:],
                                     rhs=offs_flat[0:1, tt * EL:(tt + 1) * EL],
                                     start=False, stop=True)
                    pos = pcp.tile([P, EL], f32, tag="pos")
                    nc.vector.tensor_sub(pos[:], sp[:], M_all[:, tt, :])
                    mi = pcp.tile([P, EL], u8, tag="mi")
                    nc.vector.tensor_copy(mi[:], M_all[:, tt, :])
                    big = pcp.tile([P, EL], f32, tag="big")
                    nc.vector.memset(big[:], BIGF)
                    nc.vector.copy_predicated(big[:], mi[:], pos[:])
                    nc.vector.tensor_copy(pos_all[:, tt, :], big[:])

            if debug_out:
                nc.sync.dma_start(dbg["pos"][:], pos_all[:])
                nc.sync.dma_start(dbg["cl"][:], CL_all[:])
                nc.sync.dma_start(dbg["iotas"][:], iota_s[:])
                nc.sync.dma_start(dbg["sel8"][:], sel8[:])

            # ---------------- packed pos/CL rows (partition e = expert e)
            pos_rows = cp.tile([EL, T], f16)
            cl_rows = cp.tile([EL, T], f16)
            with tc.tile_pool(name="pr", bufs=2) as prp, \
                 tc.tile_pool(name="prs", bufs=2, space="PSUM") as prs:
                for e in range(EL):
                    for (src, dst) in ((pos_all, pos_rows), (CL_all, cl_rows)):
                        col = prp.tile([P, TT], f32, tag="col")
                        nc.vector.tensor_copy(col[:], src[:, :, e])
                        tp = prs.tile([TT, P], f32, tag="tp")
                        nc.tensor.matmul(tp[:], lhsT=col[:], rhs=ident[:],
                                         start=True, stop=True)
                        tps = prp.tile([TT, P], f16, tag="tps")
                        nc.vector.tensor_copy(tps[:], tp[:])
                        nc.sync.dma_start(dst[e:e + 1, :], tps[:])

            if debug_out:
                nc.sync.dma_start(dbg["posrows"][:], pos_rows[:])
                nc.sync.dma_start(dbg["clrows"][:], cl_rows[:])

            # ---------------- Phase G: gather-mm + expert MLP -> D_e
            # ---------------- Phase S: combine-mm, dense y write
            with tc.tile_pool(name="dD", bufs=1) as dpool:
                D = [dpool.tile([P, SB, H], f16, name=f"D{e}")
                     for e in range(EL)]
                with tc.tile_pool(name="gx", bufs=2) as gx, \
                     tc.tile_pool(name="go", bufs=2) as go, \
                     tc.tile_pool(name="gxt", bufs=1) as gxt, \
                     tc.tile_pool(name="gh", bufs=1) as gh, \
                     tc.tile_pool(name="gwg", bufs=2) as gwg, \
                     tc.tile_pool(name="gwd", bufs=3) as gwd, \
                     tc.tile_pool(name="gtmp", bufs=2) as gtmp, \
                     tc.tile_pool(name="ppx", bufs=1, space="PSUM") as ppx, \
                     tc.tile_pool(name="ppg", bufs=1, space="PSUM") as ppg, \
                     tc.tile_pool(name="ppu", bufs=1, space="PSUM") as ppu, \
                     tc.tile_pool(name="ppd", bufs=1, space="PSUM") as ppd:
                    for e in range(EL):
                        # gather: XgT[h, s] = sum_t x16[t, h] * (pos[t] == s)
                        xgT = gxt.tile([P, HC, CAP], f16, tag="xgT")
                        for hg in range(HG):
                            pg0 = ppx.tile([P, CAP], f32, tag="pgh0")
                            pg1 = ppx.tile([P, CAP], f32, tag="pgh1")
                            for tt in range(TT):
                                xt = gx.tile([P, 2 * P], f16, tag="xt")
                                nc.sync.dma_start(
                                    xt[:], x16[tt * P:(tt + 1) * P,
                                               hg * 2 * P:(hg + 1) * 2 * P])
                                ot = go.tile([P, CAP], f16, tag="ot")
                                nc.vector.tensor_scalar(
                                    ot[:], iota_s[:],
                                    pos_all[:, tt, e:e + 1],
                                    scalar2=None, op0=OP.is_equal)
                                nc.tensor.matmul(pg0[:], lhsT=xt[:, 0:P],
                                                 rhs=ot[:], start=(tt == 0),
                                                 stop=(tt == TT - 1))
                                nc.tensor.matmul(pg1[:], lhsT=xt[:, P:2 * P],
                                                 rhs=ot[:], start=(tt == 0),
                                                 stop=(tt == TT - 1))
                                if debug_out and e == 0 and hg == 0 \
                                        and tt == 0:
                                    nc.sync.dma_start(dbg["o0"][:], ot[:])
                            nc.vector.tensor_copy(xgT[:, 2 * hg, :], pg0[:])
                            nc.vector.tensor_copy(xgT[:, 2 * hg + 1, :],
                                                  pg1[:])
                        if debug_out and e == 0:
                            nc.sync.dma_start(dbg["xgt0"][:], xgT[:])
                        # gate/up projections + silu*up
                        hT = gh.tile([P, IC, CAP], f16, tag="hT")
                        for i in range(IC):
                            pg = ppg.tile([P, CAP], f32, tag="pg")
                            pu = ppu.tile([P, CAP], f32, tag="pu")
                            for hf in range(2):
                                wgt = gwg.tile([P, HH, P], f16, tag="wg")
                                nc.sync.dma_start(
                                    wgt[:], wg[e, i, :, hf * HH:(hf + 1) * HH])
                                for h in range(HH):
                                    nc.tensor.matmul(
                                        pg[:], lhsT=wgt[:, h, :],
                                        rhs=xgT[:, hf * HH + h, :],
                                        start=(hf == 0 and h == 0),
                                        stop=(hf == 1 and h == HH - 1))
                            for hf in range(2):
                                wut = gwg.tile([P, HH, P], f16, tag="wu")
                                nc.sync.dma_start(
                                    wut[:], wu[e, i, :, hf * HH:(hf + 1) * HH])
                                for h in range(HH):
                                    nc.tensor.matmul(
                                        pu[:], lhsT=wut[:, h, :],
                                        rhs=xgT[:, hf * HH + h, :],
                                        start=(hf == 0 and h == 0),
                                        stop=(hf == 1 and h == HH - 1))
                            sg = gtmp.tile([P, CAP], f32, tag="sg")
                            if sim_safe:
                                nc.scalar.activation(sg[:], pg[:], AF.Sigmoid)
                                nc.vector.tensor_tensor(out=sg[:], in0=sg[:],
                                                        in1=pg[:], op=OP.mult)
                            else:
                                nc.scalar.activation(sg[:], pg[:], AF.Silu)
                            nc.vector.tensor_tensor(out=hT[:, i, :], in0=sg[:],
                                                    in1=pu[:], op=OP.mult)
                        # down projection -> D_e fp16
                        for hh in range(HB):
                            for tp_ in range(SB // 2):
                                pd0 = ppd.tile([P, 512], f32, tag="pd0")
                                pd1 = ppd.tile([P, 512], f32, tag="pd1")
                                pds = (pd0, pd1)
                                for i in range(IC):
                                    wdt = gwd.tile([P, 512], f16, tag="wd")
                                    nc.sync.dma_start(wdt[:],
                                                      wd[e, hh, :, i, :])
                                    for j in range(2):
                                        tb = tp_ * 2 + j
                                        nc.tensor.matmul(
                                            pds[j][:],
                                            lhsT=hT[:, i,
                                                    tb * P:(tb + 1) * P],
                                            rhs=wdt[:],
                                            start=(i == 0),
                                            stop=(i == IC - 1))
                                for j in range(2):
                                    tb = tp_ * 2 + j
                                    nc.vector.tensor_copy(
                                        D[e][:, tb,
                                             hh * 512:(hh + 1) * 512],
                                        pds[j][:])

                if debug_out:
                    nc.sync.dma_start(dbg["d0"][:], D[0][:])
                with tc.tile_pool(name="so", bufs=2) as so, \
                     tc.tile_pool(name="sb", bufs=2) as sbp, \
                     tc.tile_pool(name="sy", bufs=2) as syp, \
                     tc.tile_pool(name="spb", bufs=2, space="PSUM") as spb, \
                     tc.tile_pool(name="spy", bufs=2, space="PSUM") as spy:
                    for tt in range(TT):
                        otiles = []
                        for e in range(EL):
                            pb2 = spb.tile([P, P], f32, tag="pb")
                            nc.tensor.matmul(
                                pb2[:], lhsT=sel8[:, e, :],
                                rhs=pos_rows[:, tt * P:(tt + 1) * P],
                                start=True, stop=True)
                            pbs = sbp.tile([P, P], f32, tag="pbs")
                            nc.vector.tensor_copy(pbs[:], pb2[:])
                            cb2 = spb.tile([P, P], f32, tag="cb")
                            nc.tensor.matmul(
                                cb2[:], lhsT=sel8[:, e, :],
                                rhs=cl_rows[:, tt * P:(tt + 1) * P],
                                start=True, stop=True)
                            cbs = sbp.tile([P, P], f16, tag="cbs")
                            nc.vector.tensor_copy(cbs[:], cb2[:])
                            ot = so.tile([P, SB, P], f16, tag=f"sot{e}")
                            for sc in range(SB):
                                nc.vector.tensor_scalar(
                                    ot[:, sc, :], pbs[:],
                                    iota4[:, sc:sc + 1],
                                    scalar2=None, op0=OP.is_equal)
                            nc.vector.tensor_tensor(
                                out=ot[:], in0=ot[:],
                                in1=cbs[:, None, :].to_broadcast([P, SB, P]),
                                op=OP.mult)
                            if debug_out and tt == 0 and e == 0:
                                nc.sync.dma_start(dbg["ot0"][:], ot[:])
                            otiles.append(ot)
                        for hb in range(HB):
                            py = spy.tile([P, 512], f32, tag="py")
                            k = 0
                            for e in range(EL):
                                for sc in range(SB):
                                    nc.tensor.matmul(
                                        py[:], lhsT=otiles[e][:, sc, :],
                                        rhs=D[e][:, sc,
                                                 hb * 512:(hb + 1) * 512],
                                        start=(k == 0),
                                        stop=(k == EL * SB - 1))
                                    k += 1
                            ysb = syp.tile([P, 512], f32, tag="ysb")
                            nc.vector.tensor_copy(ysb[:], py[:])
                            nc.sync.dma_start(
                                y[tt * P:(tt + 1) * P,
                                  hb * 512:(hb + 1) * 512],
                                ysb[:])

    nc.compile()
    return nc


def make_in_maps(hidden_states, gate_weight, w_gate, w_up, w_down):
    x = np.ascontiguousarray(hidden_states, dtype=np.float32)
    x16 = x.astype(np.float16)
    xr = np.ascontiguousarray(
        x.reshape(TT, P, HC, P).transpose(0, 3, 2, 1))
    in_maps = []
    for c in range(NCORES):
        gwroll = np.roll(gate_weight, -EL * c, axis=0)
        gwt = np.ascontiguousarray(
            gwroll.T.reshape(HC, P, E).transpose(1, 0, 2)).astype(np.float32)
        wgs = w_gate[EL * c:EL * (c + 1)]
        wus = w_up[EL * c:EL * (c + 1)]
        wds = w_down[EL * c:EL * (c + 1)]
        wg_r = np.ascontiguousarray(
            wgs.reshape(EL, HC, P, IC, P).transpose(0, 3, 2, 1, 4)).astype(
                np.float16)
        wu_r = np.ascontiguousarray(
            wus.reshape(EL, HC, P, IC, P).transpose(0, 3, 2, 1, 4)).astype(
                np.float16)
        wd_r = np.ascontiguousarray(
            wds.reshape(EL, IC, P, HB, 512).transpose(0, 3, 2, 1, 4)).astype(
                np.float16)
        in_maps.append({
            "x16": x16, "xr": xr, "gwt": gwt,
            "wg": wg_r, "wu": wu_r, "wd": wd_r,
        })
    return in_maps


_NC_CACHE = None


def _get_nc():
    global _NC_CACHE
    if _NC_CACHE is None:
        _NC_CACHE = build_nc()
    return _NC_CACHE


def bench_hw(iters=12, pipeline_iters=16):
    """Benchmark the 8-core NEFF execute with device-resident inputs.

    Returns (t_exec, t_block_min, t_block_mean, out):
      - t_exec: steady-state per-execute time measured by submitting
        `pipeline_iters` executes back-to-back and blocking once at the end.
        Successive executes serialize on the device, so this amortizes the
        client->terminal RPC round-trip (~60ms here) out of the measurement
        and is the closest available proxy for NEFF execution time (NTFF
        profiling is unavailable in this container).
      - t_block_min/mean: one-execute-at-a-time wall-clock (includes the
        full dispatch round-trip per execute).
      - out: summed full output of the last iteration.
    """
    import time

    import jax
    import numpy as _np
    from jax.sharding import Mesh, PartitionSpec
    from jax.experimental.shard_map import shard_map

    import concourse.mybir as _mb
    from concourse import bass2jax as b2j

    nc = _get_nc()
    data = _np.load("/tmp/moe_inputs.npz")
    in_maps = make_in_maps(*[data[k] for k in
                             ("hidden_states", "gate_weight", "w_gate",
                              "w_up", "w_down")])
    b2j.install_neuronx_cc_hook()
    partition_name = (nc.partition_id_tensor.name
                      if nc.partition_id_tensor else None)
    in_names, out_names, out_avals, zero_outs = [], [], [], []
    for alloc in nc.m.functions[0].allocations:
        if not isinstance(alloc, _mb.MemoryLocationSet):
            continue
        name = alloc.memorylocations[0].name
        if alloc.kind == "ExternalInput":
            if name != partition_name:
                in_names.append(name)
        elif alloc.kind == "ExternalOutput":
            shape = tuple(alloc.tensor_shape)
            dtype = _mb.dt.np(alloc.dtype)
            out_names.append(name)
            out_avals.append(jax.core.ShapedArray(shape, dtype))
            zero_outs.append(_np.zeros(shape, dtype))
    n_params = len(in_names)
    all_in_names = list(in_names) + list(out_names)
    if partition_name is not None:
        all_in_names.append(partition_name)

    def _body(*args):
        operands = list(args)
        if partition_name is not None:
            operands.append(b2j.partition_id_tensor())
        outs = b2j._bass_exec_p.bind(
            *operands, out_avals=tuple(out_avals),
            in_names=tuple(all_in_names), out_names=tuple(out_names),
            lowering_input_output_aliases=(), sim_require_finite=True,
            sim_require_nnan=True, nc=nc)
        return tuple(outs)

    devices = jax.devices()[:NCORES]
    mesh = Mesh(_np.asarray(devices), ("core",))
    n_outs = len(out_names)
    sharded = jax.jit(shard_map(
        _body, mesh=mesh,
        in_specs=(PartitionSpec("core"),) * (n_params + n_outs),
        out_specs=(PartitionSpec("core"),) * n_outs, check_rep=False))
    concat_in = [_np.concatenate([_np.asarray(in_maps[c][nm])
                                  for c in range(NCORES)], axis=0)
                 for nm in in_names]
    concat_zeros = [_np.zeros((NCORES * z.shape[0], *z.shape[1:]), z.dtype)
                    for z in zero_outs]
    dev_in = [jax.device_put(a) for a in concat_in + concat_zeros]
    out = sharded(*dev_in)
    jax.block_until_ready(out)
    times = []
    for _ in range(iters):
        t0 = time.perf_counter()
        out = sharded(*dev_in)
        jax.block_until_ready(out)
        times.append(time.perf_counter() - t0)
    # pipelined steady-state: submit back-to-back, block once
    t0 = time.perf_counter()
    outs = [sharded(*dev_in) for _ in range(pipeline_iters)]
    jax.block_until_ready(outs[-1])
    t_exec = (time.perf_counter() - t0) / pipeline_iters
    out = outs[-1]
    yfull = _np.asarray(out[out_names.index("y")]).reshape(
        NCORES, T, H).sum(axis=0)
    return t_exec, min(times), sum(times) / len(times), yfull


LAST_RESULTS = None


def kernel(hidden_states, gate_weight, w_gate, w_up, w_down):
    global LAST_RESULTS
    nc = _get_nc()
    in_maps = make_in_maps(np.asarray(hidden_states), np.asarray(gate_weight),
                           np.asarray(w_gate), np.asarray(w_up),
                           np.asarray(w_down))
    trace = bool(int(os.environ.get("MOE_TRACE", "0")))
    res = run_bass_kernel_spmd(
        nc, in_maps, core_ids=list(range(NCORES)), trace=trace,
        trace_cores=list(range(NCORES)) if trace else None)
    LAST_RESULTS = res
    out = np.zeros((T, H), dtype=np.float32)
    for r in res.results:
        out += r["y"]
    return out


# revision 3
# speedup vs baseline: 1.1486x; 1.1486x over previous
"""DeepSeek-MoE (64 experts, top-6 grouped routing) on 8 TRN2 NeuronCores.

Expert-parallel, v2: all dispatch/combine is done with one-hot MATMULS on
the PE — no indirect DMA at all (v1's dma_gather / dma_scatter_add /
per-column indirect scatters generated ~40K software-DGE descriptors which
dominated hardware time at ~µs each).

  - Every core receives the full hidden_states (fp32 rearranged for the
    router, fp16 row-major for the expert GEMMs) plus an 8-expert shard of
    w_gate/w_up/w_down (fp16) and a group-rotated gate matrix so that its
    local experts always occupy routing columns 0..7.
  - Router (fp32): logits -> softmax -> grouped top-6 -> renormalized
    combine weights CL, mask M, and per-(token,expert) slot index pos
    (prefix-sum matmuls).
  - Gather: for each local expert e, one-hot O_tt[t, s] = (pos[t]==s) is
    built on the DVE and XgT[h, s] = sum_t x[t, h] * O[t, s] accumulates on
    the PE straight into the [H-part, slot] layout the MLP wants.
  - MLP: fp16 matmuls with fp32 PSUM accumulation (gate/up -> silu*up ->
    down), producing D_e[slot, h] fp16. Combine weights are NOT folded here.
  - Combine: OT_w[s, t] = (pos[t]==s) * CL[t] is built per token tile from
    broadcast rows, and Y[t, h] = sum_s OT_w[s, t] * D[s, h] accumulates on
    the PE over all 8 experts; y is written densely (no scatter).
  - Host sums the 8 partial outputs.

Empty slots produce exact zeros end-to-end (O column zero -> Xg zero ->
silu(0)*0 = 0 -> D zero), and unrouted tokens carry pos=100000 which never
matches a slot, so no counts/registers/bounds checks are needed.
"""

import os

import numpy as np

import concourse.bacc as bacc
import concourse.bass as bass
import concourse.mybir as mybir
import concourse.tile as tile
from concourse.bass_utils import run_bass_kernel_spmd
from concourse.masks import make_identity, make_upper_triangular

P = 128
T = 4096          # tokens
H = 2048          # hidden
ID = 1408         # intermediate
E = 64            # experts
EL = 8            # local experts per core
NCORES = 8
CAP = 512         # per-expert token capacity (actual max count is ~454)
TT = T // P       # 32 token tiles
HC = H // P       # 16 hidden chunks
IC = ID // P      # 11 intermediate chunks
HB = H // 512     # 4 hidden blocks (down-proj rhs width 512)
SB = CAP // P     # 4 slot blocks per expert
HG = 8            # gather h-groups (2 psum banks each)
HH = HC // 2      # half of the hidden chunks (gate/up weight half-tiles)
BIGF = 30000.0    # unrouted-token pos marker: never matches a slot
                  # (finite in fp16: 0*inf=NaN would poison the
                  #  sel8 broadcast matmul over pos_rows)

f32 = mybir.dt.float32
f16 = mybir.dt.float16
i32 = mybir.dt.int32
u8 = mybir.dt.uint8
AF = mybir.ActivationFunctionType
OP = mybir.AluOpType
AX = mybir.AxisListType


def build_nc(debug=False, sim_safe=False, debug_out=False):
    nc = bacc.Bacc("TRN2", target_bir_lowering=False, debug=debug)

    dbg = {}
    if debug_out:
        dbg["pos"] = nc.dram_tensor("dbg_pos", [P, TT, EL], f32,
                                    kind="ExternalOutput")
        dbg["cl"] = nc.dram_tensor("dbg_cl", [P, TT, EL], f32,
                                   kind="ExternalOutput")
        dbg["posrows"] = nc.dram_tensor("dbg_posrows", [EL, T], f16,
                                        kind="ExternalOutput")
        dbg["clrows"] = nc.dram_tensor("dbg_clrows", [EL, T], f16,
                                       kind="ExternalOutput")
        dbg["iotas"] = nc.dram_tensor("dbg_iotas", [P, CAP], f16,
                                      kind="ExternalOutput")
        dbg["sel8"] = nc.dram_tensor("dbg_sel8", [EL, EL, P], f16,
                                     kind="ExternalOutput")
        dbg["o0"] = nc.dram_tensor("dbg_o0", [P, CAP], f16,
                                   kind="ExternalOutput")
        dbg["xgt0"] = nc.dram_tensor("dbg_xgt0", [P, HC, CAP], f16,
                                     kind="ExternalOutput")
        dbg["d0"] = nc.dram_tensor("dbg_d0", [P, SB, H], f16,
                                   kind="ExternalOutput")
        dbg["ot0"] = nc.dram_tensor("dbg_ot0", [P, SB, P], f16,
                                    kind="ExternalOutput")

    x16 = nc.dram_tensor("x16", [T, H], f16, kind="ExternalInput")
    xr = nc.dram_tensor("xr", [TT, P, HC, P], f32, kind="ExternalInput")
    gwt = nc.dram_tensor("gwt", [P, HC, E], f32, kind="ExternalInput")
    wg = nc.dram_tensor("wg", [EL, IC, P, HC, P], f16, kind="ExternalInput")
    wu = nc.dram_tensor("wu", [EL, IC, P, HC, P], f16, kind="ExternalInput")
    wd = nc.dram_tensor("wd", [EL, HB, P, IC, 512], f16, kind="ExternalInput")
    y = nc.dram_tensor("y", [T, H], f16, kind="ExternalOutput")

    with tile.TileContext(nc) as tc:
        with tc.tile_pool(name="const", bufs=1) as cp:
            ident = cp.tile([P, P], f32)
            make_identity(nc, ident[:])
            ut = cp.tile([P, P], f32)
            make_upper_triangular(nc, ut[:], val=1.0, diag=True)
            sut = cp.tile([TT, TT], f32)
            make_upper_triangular(nc, sut[:], val=1.0, diag=False)
            onesk = cp.tile([P, 1], f32)
            nc.vector.memset(onesk[:], 1.0)
            ones1 = cp.tile([1, P], f32)
            nc.vector.memset(ones1[:], 1.0)
            M_all = cp.tile([P, TT, EL], f32)
            CL_all = cp.tile([P, TT, EL], f32)     # combine weights
            offs_flat = cp.tile([1, TT * EL], f32)
            tot32 = cp.tile([TT, EL], f32)
            # iota row: [p, s] = s, fp16
            iota_s_i = cp.tile([P, CAP], i32)
            nc.gpsimd.iota(iota_s_i[:], pattern=[[1, CAP]], base=0,
                           channel_multiplier=0)
            iota_s = cp.tile([P, CAP], f16)
            nc.vector.tensor_copy(iota_s[:], iota_s_i[:])
            # iota cols: [p, sc] = p + 128*sc, fp16
            iota4_i = cp.tile([P, SB], i32)
            nc.gpsimd.iota(iota4_i[:], pattern=[[P, SB]], base=0,
                           channel_multiplier=1)
            iota4 = cp.tile([P, SB], f32)
            nc.vector.tensor_copy(iota4[:], iota4_i[:])
            # sel8[p, e, i] = 1 iff p == e (expert-row selector for
            # broadcast matmuls from the packed row tiles)
            iota_e_i = cp.tile([EL, EL], i32)
            nc.gpsimd.iota(iota_e_i[:], pattern=[[1, EL]], base=0,
                           channel_multiplier=0)
            iota_e = cp.tile([EL, EL], f16)
            nc.vector.tensor_copy(iota_e[:], iota_e_i[:])
            iota_p_i = cp.tile([EL, 1], i32)
            nc.gpsimd.iota(iota_p_i[:], pattern=[[0, 1]], base=0,
                           channel_multiplier=1)
            iota_p = cp.tile([EL, 1], f32)
            nc.vector.tensor_copy(iota_p[:], iota_p_i[:])
            sel8 = cp.tile([EL, EL, P], f16)
            nc.vector.tensor_copy(
                sel8[:], iota_e[:, :, None].to_broadcast([EL, EL, P]))
            nc.vector.tensor_scalar(sel8[:], sel8[:], iota_p[:, 0:1],
                                    scalar2=None, op0=OP.is_equal)

            # ---------------- Phase A: router over all 32 token tiles
            with tc.tile_pool(name="ra", bufs=3) as ra, \
                 tc.tile_pool(name="rg", bufs=1) as rg, \
                 tc.tile_pool(name="rp", bufs=2, space="PSUM") as rp:
                gwt_sb = rg.tile([P, HC, E], f32)
                nc.sync.dma_start(gwt_sb[:], gwt[:])
                for tt in range(TT):
                    xrt = ra.tile([P, HC, P], f32, tag="xrt")
                    nc.sync.dma_start(xrt[:], xr[tt])
                    psl = rp.tile([P, E], f32, tag="psl")
                    for h in range(HC):
                        nc.tensor.matmul(psl[:], lhsT=xrt[:, h, :],
                                         rhs=gwt_sb[:, h, :],
                                         start=(h == 0), stop=(h == HC - 1))
                    nrm = ra.tile([P, 1], f32, tag="nrm")
                    nc.vector.tensor_reduce(out=nrm[:], in_=psl[:], axis=AX.X,
                                            op=OP.max, negate=True)
                    expt = ra.tile([P, E], f32, tag="expt")
                    nc.scalar.activation(expt[:], psl[:], AF.Exp, bias=nrm[:])
                    gs = ra.tile([P, 8], f32, tag="gs")
                    nc.vector.tensor_reduce(
                        out=gs[:], in_=expt[:].rearrange("p (g k) -> p g k", g=8),
                        axis=AX.X, op=OP.max)
                    g8 = ra.tile([P, 8], f32, tag="g8")
                    nc.vector.max(out=g8[:], in_=gs[:])
                    g3 = ra.tile([P, 8], f32, tag="g3")
                    nc.vector.tensor_copy(g3[:], g8[:])
                    nc.vector.memset(g3[:, 3:8], 0.0)
                    gsr = ra.tile([P, 8], f32, tag="gsr")
                    nc.vector.match_replace(out=gsr[:], in_to_replace=g3[:],
                                            in_values=gs[:], imm_value=0.0)
                    gm = ra.tile([P, 8], f32, tag="gm")
                    nc.vector.tensor_sub(gm[:], gs[:], gsr[:])
                    nc.vector.tensor_scalar(gm[:], gm[:], 0.0, scalar2=None,
                                            op0=OP.is_gt)
                    msk = ra.tile([P, E], f32, tag="msk")
                    nc.vector.tensor_tensor(
                        out=msk[:].rearrange("p (g k) -> p g k", g=8),
                        in0=expt[:].rearrange("p (g k) -> p g k", g=8),
                        in1=gm[:, :, None].to_broadcast([P, 8, 8]),
                        op=OP.mult)
                    m8 = ra.tile([P, 8], f32, tag="m8")
                    nc.vector.max(out=m8[:], in_=msk[:])
                    m6 = ra.tile([P, 8], f32, tag="m6")
                    nc.vector.tensor_copy(m6[:], m8[:])
                    nc.vector.memset(m6[:, 6:8], -1.0)
                    rem = ra.tile([P, E], f32, tag="rem")
                    nc.vector.match_replace(out=rem[:], in_to_replace=m6[:],
                                            in_values=msk[:], imm_value=0.0)
                    sel = ra.tile([P, E], f32, tag="sel")
                    nc.vector.tensor_sub(sel[:], msk[:], rem[:])
                    rs = ra.tile([P, 1], f32, tag="rs")
                    nc.vector.tensor_reduce(out=rs[:], in_=sel[:], axis=AX.X,
                                            op=OP.add)
                    nc.vector.tensor_scalar(rs[:], rs[:], 1e-20, scalar2=None,
                                            op0=OP.add)
                    rinv = ra.tile([P, 1], f32, tag="rinv")
                    nc.vector.reciprocal(rinv[:], rs[:])
                    cl = ra.tile([P, EL], f32, tag="cl")
                    nc.vector.tensor_scalar(cl[:], sel[:, 0:EL], rinv[:],
                                            scalar2=None, op0=OP.mult)
                    nc.vector.tensor_copy(CL_all[:, tt, :], cl[:])
                    nc.vector.tensor_scalar(M_all[:, tt, :], cl[:], 0.0,
                                            scalar2=None, op0=OP.is_gt)

            # ---------------- Phase B: totals -> per-tile offsets
            with tc.tile_pool(name="pb", bufs=1) as pb, \
                 tc.tile_pool(name="pbp", bufs=1, space="PSUM") as pbp:
                totp = pbp.tile([1, TT * EL], f32)
                nc.tensor.matmul(totp[:], lhsT=onesk[:],
                                 rhs=M_all[:].rearrange("p t e -> p (t e)"),
                                 start=True, stop=True)
                tots = pb.tile([1, TT * EL], f32)
                nc.vector.tensor_copy(tots[:], totp[:])
                nc.sync.dma_start(tot32[:], tots[:])
                offp = pbp.tile([TT, EL], f32)
                nc.tensor.matmul(offp[:], lhsT=sut[:], rhs=tot32[:],
                                 start=True, stop=True)
                offs32 = pb.tile([TT, EL], f32)
                nc.vector.tensor_copy(offs32[:], offp[:])
                nc.sync.dma_start(offs_flat[:], offs32[:])

            # ---------------- Phase C: slot index pos per (token, expert)
            pos_all = cp.tile([P, TT, EL], f32)
            with tc.tile_pool(name="pc", bufs=3) as pcp, \
                 tc.tile_pool(name="pcs", bufs=2, space="PSUM") as pcs:
                for tt in range(TT):
                    sp = pcs.tile([P, EL], f32, tag="sp")
                    nc.tensor.matmul(sp[:], lhsT=ut[:], rhs=M_all[:, tt, :],
                                     start=True, stop=False)
                    nc.tensor.matmul(sp[:], lhsT=ones1[:],
                                     rhs=offs_flat[0:1, tt * EL:(tt + 1) * EL],
                                     start=False, stop=True)
                    pos = pcp.tile([P, EL], f32, tag="pos")
                    nc.vector.tensor_sub(pos[:], sp[:], M_all[:, tt, :])
                    mi = pcp.tile([P, EL], u8, tag="mi")
                    nc.vector.tensor_copy(mi[:], M_all[:, tt, :])
                    big = pcp.tile([P, EL], f32, tag="big")
                    nc.vector.memset(big[:], BIGF)
                    nc.vector.copy_predicated(big[:], mi[:], pos[:])
                    nc.vector.tensor_copy(pos_all[:, tt, :], big[:])

            if debug_out:
                nc.sync.dma_start(dbg["pos"][:], pos_all[:])
                nc.sync.dma_start(dbg["cl"][:], CL_all[:])
                nc.sync.dma_start(dbg["iotas"][:], iota_s[:])
                nc.sync.dma_start(dbg["sel8"][:], sel8[:])

            # ---------------- packed pos/CL rows (partition e = expert e)
            pos_rows = cp.tile([EL, T], f16)
            cl_rows = cp.tile([EL, T], f16)
            with tc.tile_pool(name="pr", bufs=2) as prp, \
                 tc.tile_pool(name="prs", bufs=2, space="PSUM") as prs:
                for e in range(EL):
                    for (src, dst) in ((pos_all, pos_rows), (CL_all, cl_rows)):
                        col = prp.tile([P, TT], f32, tag="col")
                        nc.vector.tensor_copy(col[:], src[:, :, e])
                        tp = prs.tile([TT, P], f32, tag="tp")
                        nc.tensor.matmul(tp[:], lhsT=col[:], rhs=ident[:],
                                         start=True, stop=True)
                        tps = prp.tile([TT, P], f16, tag="tps")
                        nc.vector.tensor_copy(tps[:], tp[:])
                        nc.sync.dma_start(dst[e:e + 1, :], tps[:])

            if debug_out:
                nc.sync.dma_start(dbg["posrows"][:], pos_rows[:])
                nc.sync.dma_start(dbg["clrows"][:], cl_rows[:])

            # ---------------- Phase G: gather-mm + expert MLP -> D_e
            # ---------------- Phase S: combine-mm, dense y write
            with tc.tile_pool(name="dD", bufs=1) as dpool:
                D = [dpool.tile([P, SB, H], f16, name=f"D{e}")
                     for e in range(EL)]
                with tc.tile_pool(name="gx", bufs=2) as gx, \
                     tc.tile_pool(name="go", bufs=2) as go, \
                     tc.tile_pool(name="gxt", bufs=1) as gxt, \
                     tc.tile_pool(name="gh", bufs=1) as gh, \
                     tc.tile_pool(name="gwg", bufs=2) as gwg, \
                     tc.tile_pool(name="gwd", bufs=3) as gwd, \
                     tc.tile_pool(name="gtmp", bufs=2) as gtmp, \
                     tc.tile_pool(name="ppx", bufs=1, space="PSUM") as ppx, \
                     tc.tile_pool(name="ppg", bufs=1, space="PSUM") as ppg, \
                     tc.tile_pool(name="ppu", bufs=1, space="PSUM") as ppu, \
                     tc.tile_pool(name="ppd", bufs=1, space="PSUM") as ppd:
                    for e in range(EL):
                        # gather: XgT[h, s] = sum_t x16[t, h] * (pos[t] == s)
                        xgT = gxt.tile([P, HC, CAP], f16, tag="xgT")
                        for hg in range(HG):
                            pg0 = ppx.tile([P, CAP], f32, tag="pgh0")
                            pg1 = ppx.tile([P, CAP], f32, tag="pgh1")
                            for tt in range(TT):
                                xt = gx.tile([P, 2 * P], f16, tag="xt")
                                nc.sync.dma_start(
                                    xt[:], x16[tt * P:(tt + 1) * P,
                                               hg * 2 * P:(hg + 1) * 2 * P])
                                ot = go.tile([P, CAP], f16, tag="ot")
                                nc.vector.tensor_scalar(
                                    ot[:], iota_s[:],
                                    pos_all[:, tt, e:e + 1],
                                    scalar2=None, op0=OP.is_equal)
                                nc.tensor.matmul(pg0[:], lhsT=xt[:, 0:P],
                                                 rhs=ot[:], start=(tt == 0),
                                                 stop=(tt == TT - 1))
                                nc.tensor.matmul(pg1[:], lhsT=xt[:, P:2 * P],
                                                 rhs=ot[:], start=(tt == 0),
                                                 stop=(tt == TT - 1))
                                if debug_out and e == 0 and hg == 0 \
                                        and tt == 0:
                                    nc.sync.dma_start(dbg["o0"][:], ot[:])
                            nc.vector.tensor_copy(xgT[:, 2 * hg, :], pg0[:])
                            nc.vector.tensor_copy(xgT[:, 2 * hg + 1, :],
                                                  pg1[:])
                        if debug_out and e == 0:
                            nc.sync.dma_start(dbg["xgt0"][:], xgT[:])
                        # gate/up projections + silu*up
                        hT = gh.tile([P, IC, CAP], f16, tag="hT")
                        for i in range(IC):
                            pg = ppg.tile([P, CAP], f32, tag="pg")
                            pu = ppu.tile([P, CAP], f32, tag="pu")
                            for hf in range(2):
                                wgt = gwg.tile([P, HH, P], f16, tag="wg")
                                nc.sync.dma_start(
                                    wgt[:], wg[e, i, :, hf * HH:(hf + 1) * HH])
                                for h in range(HH):
                                    nc.tensor.matmul(
                                        pg[:], lhsT=wgt[:, h, :],
                                        rhs=xgT[:, hf * HH + h, :],
                                        start=(hf == 0 and h == 0),
                                        stop=(hf == 1 and h == HH - 1))
                            for hf in range(2):
                                wut = gwg.tile([P, HH, P], f16, tag="wu")
                                nc.sync.dma_start(
                                    wut[:], wu[e, i, :, hf * HH:(hf + 1) * HH])
                                for h in range(HH):
                                    nc.tensor.matmul(
                                        pu[:], lhsT=wut[:, h, :],
                                        rhs=xgT[:, hf * HH + h, :],
                                        start=(hf == 0 and h == 0),
                                        stop=(hf == 1 and h == HH - 1))
                            sg = gtmp.tile([P, CAP], f32, tag="sg")
                            if sim_safe:
                                nc.scalar.activation(sg[:], pg[:], AF.Sigmoid)
                                nc.vector.tensor_tensor(out=sg[:], in0=sg[:],
                                                        in1=pg[:], op=OP.mult)
                            else:
                                nc.scalar.activation(sg[:], pg[:], AF.Silu)
                            nc.vector.tensor_tensor(out=hT[:, i, :], in0=sg[:],
                                                    in1=pu[:], op=OP.mult)
                        # down projection -> D_e fp16
                        for hh in range(HB):
                            for tp_ in range(SB // 2):
                                pd0 = ppd.tile([P, 512], f32, tag="pd0")
                                pd1 = ppd.tile([P, 512], f32, tag="pd1")
                                pds = (pd0, pd1)
                                for i in range(IC):
                                    wdt = gwd.tile([P, 512], f16, tag="wd")
                                    nc.sync.dma_start(wdt[:],
                                                      wd[e, hh, :, i, :])
                                    for j in range(2):
                                        tb = tp_ * 2 + j
                                        nc.tensor.matmul(
                                            pds[j][:],
                                            lhsT=hT[:, i,
                                                    tb * P:(tb + 1) * P],
                                            rhs=wdt[:],
                                            start=(i == 0),
                                            stop=(i == IC - 1))
                                for j in range(2):
                                    tb = tp_ * 2 + j
                                    nc.vector.tensor_copy(
                                        D[e][:, tb,
                                             hh * 512:(hh + 1) * 512],
                                        pds[j][:])

                if debug_out:
                    nc.sync.dma_start(dbg["d0"][:], D[0][:])
                with tc.tile_pool(name="so", bufs=2) as so, \
                     tc.tile_pool(name="sb", bufs=2) as sbp, \
                     tc.tile_pool(name="sy", bufs=2) as syp, \
                     tc.tile_pool(name="spb", bufs=2, space="PSUM") as spb, \
                     tc.tile_pool(name="spy", bufs=2, space="PSUM") as spy:
                    for tt in range(TT):
                        otiles = []
                        for e in range(EL):
                            pb2 = spb.tile([P, P], f32, tag="pb")
                            nc.tensor.matmul(
                                pb2[:], lhsT=sel8[:, e, :],
                                rhs=pos_rows[:, tt * P:(tt + 1) * P],
                                start=True, stop=True)
                            pbs = sbp.tile([P, P], f32, tag="pbs")
                            nc.vector.tensor_copy(pbs[:], pb2[:])
                            cb2 = spb.tile([P, P], f32, tag="cb")
                            nc.tensor.matmul(
                                cb2[:], lhsT=sel8[:, e, :],
                                rhs=cl_rows[:, tt * P:(tt + 1) * P],
                                start=True, stop=True)
                            cbs = sbp.tile([P, P], f16, tag="cbs")
                            nc.vector.tensor_copy(cbs[:], cb2[:])
                            ot = so.tile([P, SB, P], f16, tag=f"sot{e}")
                            for sc in range(SB):
                                nc.vector.tensor_scalar(
                                    ot[:, sc, :], pbs[:],
                                    iota4[:, sc:sc + 1],
                                    scalar2=None, op0=OP.is_equal)
                            nc.vector.tensor_tensor(
                                out=ot[:], in0=ot[:],
                                in1=cbs[:, None, :].to_broadcast([P, SB, P]),
                                op=OP.mult)
                            if debug_out and tt == 0 and e == 0:
                                nc.sync.dma_start(dbg["ot0"][:], ot[:])
                            otiles.append(ot)
                        for hb in range(HB):
                            py = spy.tile([P, 512], f32, tag="py")
                            k = 0
                            for e in range(EL):
                                for sc in range(SB):
                                    nc.tensor.matmul(
                                        py[:], lhsT=otiles[e][:, sc, :],
                                        rhs=D[e][:, sc,
                                                 hb * 512:(hb + 1) * 512],
                                        start=(k == 0),
                                        stop=(k == EL * SB - 1))
                                    k += 1
                            ysb = syp.tile([P, 512], f16, tag="ysb")
                            nc.vector.tensor_copy(ysb[:], py[:])
                            nc.sync.dma_start(
                                y[tt * P:(tt + 1) * P,
                                  hb * 512:(hb + 1) * 512],
                                ysb[:])

    nc.compile()
    return nc


def make_in_maps(hidden_states, gate_weight, w_gate, w_up, w_down):
    x = np.ascontiguousarray(hidden_states, dtype=np.float32)
    x16 = x.astype(np.float16)
    xr = np.ascontiguousarray(
        x.reshape(TT, P, HC, P).transpose(0, 3, 2, 1))
    in_maps = []
    for c in range(NCORES):
        gwroll = np.roll(gate_weight, -EL * c, axis=0)
        gwt = np.ascontiguousarray(
            gwroll.T.reshape(HC, P, E).transpose(1, 0, 2)).astype(np.float32)
        wgs = w_gate[EL * c:EL * (c + 1)]
        wus = w_up[EL * c:EL * (c + 1)]
        wds = w_down[EL * c:EL * (c + 1)]
        wg_r = np.ascontiguousarray(
            wgs.reshape(EL, HC, P, IC, P).transpose(0, 3, 2, 1, 4)).astype(
                np.float16)
        wu_r = np.ascontiguousarray(
            wus.reshape(EL, HC, P, IC, P).transpose(0, 3, 2, 1, 4)).astype(
                np.float16)
        wd_r = np.ascontiguousarray(
            wds.reshape(EL, IC, P, HB, 512).transpose(0, 3, 2, 1, 4)).astype(
                np.float16)
        in_maps.append({
            "x16": x16, "xr": xr, "gwt": gwt,
            "wg": wg_r, "wu": wu_r, "wd": wd_r,
        })
    return in_maps


_NC_CACHE = None


def _get_nc():
    global _NC_CACHE
    if _NC_CACHE is None:
        _NC_CACHE = build_nc()
    return _NC_CACHE


def bench_hw(iters=12, pipeline_iters=16):
    """Benchmark the 8-core NEFF execute with device-resident inputs.

    Returns (t_exec, t_block_min, t_block_mean, out):
      - t_exec: steady-state per-execute time measured by submitting
        `pipeline_iters` executes back-to-back and blocking once at the end.
        Successive executes serialize on the device, so this amortizes the
        client->terminal RPC round-trip (~60ms here) out of the measurement
        and is the closest available proxy for NEFF execution time (NTFF
        profiling is unavailable in this container).
      - t_block_min/mean: one-execute-at-a-time wall-clock (includes the
        full dispatch round-trip per execute).
      - out: summed full output of the last iteration.
    """
    import time

    import jax
    import numpy as _np
    from jax.sharding import Mesh, PartitionSpec
    from jax.experimental.shard_map import shard_map

    import concourse.mybir as _mb
    from concourse import bass2jax as b2j

    nc = _get_nc()
    data = _np.load("/tmp/moe_inputs.npz")
    in_maps = make_in_maps(*[data[k] for k in
                             ("hidden_states", "gate_weight", "w_gate",
                              "w_up", "w_down")])
    b2j.install_neuronx_cc_hook()
    partition_name = (nc.partition_id_tensor.name
                      if nc.partition_id_tensor else None)
    in_names, out_names, out_avals, zero_outs = [], [], [], []
    for alloc in nc.m.functions[0].allocations:
        if not isinstance(alloc, _mb.MemoryLocationSet):
            continue
        name = alloc.memorylocations[0].name
        if alloc.kind == "ExternalInput":
            if name != partition_name:
                in_names.append(name)
        elif alloc.kind == "ExternalOutput":
            shape = tuple(alloc.tensor_shape)
            dtype = _mb.dt.np(alloc.dtype)
            out_names.append(name)
            out_avals.append(jax.core.ShapedArray(shape, dtype))
            zero_outs.append(_np.zeros(shape, dtype))
    n_params = len(in_names)
    all_in_names = list(in_names) + list(out_names)
    if partition_name is not None:
        all_in_names.append(partition_name)

    def _body(*args):
        operands = list(args)
        if partition_name is not None:
            operands.append(b2j.partition_id_tensor())
        outs = b2j._bass_exec_p.bind(
            *operands, out_avals=tuple(out_avals),
            in_names=tuple(all_in_names), out_names=tuple(out_names),
            lowering_input_output_aliases=(), sim_require_finite=True,
            sim_require_nnan=True, nc=nc)
        return tuple(outs)

    devices = jax.devices()[:NCORES]
    mesh = Mesh(_np.asarray(devices), ("core",))
    n_outs = len(out_names)
    sharded = jax.jit(shard_map(
        _body, mesh=mesh,
        in_specs=(PartitionSpec("core"),) * (n_params + n_outs),
        out_specs=(PartitionSpec("core"),) * n_outs, check_rep=False))
    concat_in = [_np.concatenate([_np.asarray(in_maps[c][nm])
                                  for c in range(NCORES)], axis=0)
                 for nm in in_names]
    concat_zeros = [_np.zeros((NCORES * z.shape[0], *z.shape[1:]), z.dtype)
                    for z in zero_outs]
    dev_in = [jax.device_put(a) for a in concat_in + concat_zeros]
    out = sharded(*dev_in)
    jax.block_until_ready(out)
    times = []
    for _ in range(iters):
        t0 = time.perf_counter()
        out = sharded(*dev_in)
        jax.block_until_ready(out)
        times.append(time.perf_counter() - t0)
    # pipelined steady-state: submit back-to-back, block once
    t0 = time.perf_counter()
    outs = [sharded(*dev_in) for _ in range(pipeline_iters)]
    jax.block_until_ready(outs[-1])
    t_exec = (time.perf_counter() - t0) / pipeline_iters
    out = outs[-1]
    yfull = _np.asarray(out[out_names.index("y")]).reshape(
        NCORES, T, H).astype(_np.float32).sum(axis=0)
    return t_exec, min(times), sum(times) / len(times), yfull


LAST_RESULTS = None


def kernel(hidden_states, gate_weight, w_gate, w_up, w_down):
    global LAST_RESULTS
    nc = _get_nc()
    in_maps = make_in_maps(np.asarray(hidden_states), np.asarray(gate_weight),
                           np.asarray(w_gate), np.asarray(w_up),
                           np.asarray(w_down))
    trace = bool(int(os.environ.get("MOE_TRACE", "0")))
    res = run_bass_kernel_spmd(
        nc, in_maps, core_ids=list(range(NCORES)), trace=trace,
        trace_cores=list(range(NCORES)) if trace else None)
    LAST_RESULTS = res
    out = np.zeros((T, H), dtype=np.float32)
    for r in res.results:
        out += r["y"].astype(np.float32)
    return out
